# revision 7
# baseline (speedup 1.0000x reference)
"""BitLinear forward on 8 Trainium2 NeuronCores.

Sharding: 2-way data parallel over tokens x 4-way tensor parallel over
output features. Each core computes y[token_half, out_quarter] =
[4096, 1024] of the full [8192, 4096] output.

Per-core pipeline (all on-device), engine-balanced so the TensorE is
the bottleneck:
  W phase (per 128-row tile, 8 tiles): stream W [128, 4096] f32 in one
    DMA; DVE abs-sum reduce -> w_scale; ACT magic-round (scale=1/ws,
    bias=+M) then ACT Sign(x - M) -> exact ternary fp16 (clip(round(v))
    == sign(round(v)) for ternary). DVE reduce -> colsum (for the fp16
    magic-offset correction). w_scale*alpha and 1536*colsum rows bounce
    through DRAM and broadcast-load as [128, 512] tiles. DMA-transpose
    ternary weights into wqt (contraction dim on partitions).
  A phase (per 128-token tile, 32 tiles): one 2 MiB DMA, one DVE
    abs-max reduce -> a_scale, ACT magic-round to a+M (f32), ACT rebias
    by (1536 - M) -> fp16 holding exactly a_quant + 1536 (integers
    <= 1663, exact in fp16 -- the +1536 offset folds the de-magic step
    into the per-output-channel correction). DMA-transpose, then 32
    accumulating fp16 matmuls per 512-wide PSUM chunk.
  Epilogue: DVE subtracts the 1536*colsum broadcast row (exact integer
    y), ACT scales by a_scale/127 per token, DVE multiplies by the
    w_scale*alpha broadcast row; one 0.5 MiB DMA out per token tile.

All wide elementwise ops run on ACT (activation) or DVE tensor_tensor /
tensor_reduce at ~1 elem/cycle; wide tensor_scalar on DVE/GpSimd (~14
cycles/elem on this hw) is avoided entirely. DMA-transpose descriptor
generation (~5us per issue) runs on GpSimd (aq) and Sync (wq), keeping
ACT free.

The quantized operands are exact in fp16 (integers <= 1663, ternary
weights), so the fp16 matmul is bit-exact integer arithmetic.
"""
import sys

sys.path.insert(0, "/opt/trn_rl_repo")

import numpy as np

B, S, DI, DOUT = 4, 2048, 4096, 4096
DP, TP = 2, 4
T_C = B * S // DP      # 4096 tokens per core
O_C = DOUT // TP       # 1024 out features per core
NT = T_C // 128        # 32 token tiles
NJ = O_C // 128        # 8 weight row tiles
KL = DI // 128         # 32 k-slices
OCW = 512              # output chunk width (one PSUM bank)
NOC = O_C // OCW       # 2 output chunks

EPS = 1e-8
QMAX = 127.0
MAGIC = 12582912.0     # 1.5 * 2**23; f32 add rounds to nearest-even int
HOFF = 1536.0          # fp16 offset: a+1536 in [1409,1663], exact in fp16

_cached = {}


def _install_walrus_workarounds(tile_mod, mybir):
    """This walrus build rejects instructions with more than one sem wait
    ('Too many sync wait commands'). Split the Tile tail-drain waits over
    several sequencer drains; regular instructions are handled by
    _split_sync_waits after scheduling."""
    from concourse.vector_clock import ScopedClock

    def _drain_and_barrier_split(self, tick_clock, wait_clock):
        drain_inst = self.nc.sync.drain()
        wait_clock.add_sem_waits(
            drain_inst.ins, ScopedClock({None: tick_clock.global_clock})
        )
        waits = list(drain_inst.ins.sync_info.on_wait)
        if len(waits) > 1:
            del drain_inst.ins.sync_info.on_wait[1:]
            for w in waits[1:]:
                extra = self.nc.sync.drain()
                extra.ins.sync_info = mybir.SyncInfo(on_wait=[w], on_update=[])

        self.nc.all_engine_barrier()
        assert self.sems is not None
        popped = self.nc._tile_sem_poison_stack.pop()
        assert popped is self._sem_poison
        self.nc.clear_and_free_semaphores(list(self.sems.allocated().values()))
        self.nc.all_engine_barrier()

    tile_mod.TileContext._drain_and_barrier = _drain_and_barrier_split


def _split_sync_waits(nc, mybir, max_waits=1):
    """Move excess sem waits onto same-engine NoOps inserted before the
    offending instruction (engines run their stream in order, so the wait
    conjunction is preserved)."""
    n = 0
    for fn in nc.m.functions:
        for bb in fn.blocks:
            insts = bb.instructions
            i = 0
            while i < len(insts):
                inst = insts[i]
                si = getattr(inst, "sync_info", None)
                if si is not None and si.on_wait and len(si.on_wait) > max_waits:
                    waits = list(si.on_wait)
                    extra = waits[: len(waits) - max_waits]
                    del si.on_wait[: len(waits) - max_waits]
                    nops = []
                    for j in range(0, len(extra), max_waits):
                        nop = mybir.InstNoOp(name=f"WSPLIT-{n}", ins=[], outs=[])
                        n += 1
                        nop.engine = inst.engine
                        nop.sync_info = mybir.SyncInfo(
                            on_wait=list(extra[j : j + max_waits]), on_update=[]
                        )
                        nops.append(nop)
                    insts[i:i] = nops
                    i += len(nops)
                i += 1
    return n


def _build():
    import contextlib

    import concourse.bass as bass
    import concourse.tile as tile
    from concourse import mybir

    _install_walrus_workarounds(tile, mybir)

    F32 = mybir.dt.float32
    F16 = mybir.dt.float16
    Alu = mybir.AluOpType
    Act = mybir.ActivationFunctionType
    Ax = mybir.AxisListType

    nc = bass.Bass("TRN2", target_bir_lowering=False, debug=False, num_devices=8)
    x_d = nc.declare_dram_parameter("x", [T_C, DI], F32, isOutput=False)
    w_d = nc.declare_dram_parameter("w", [O_C, DI], F32, isOutput=False)
    al_d = nc.declare_dram_parameter("alpha", [O_C], F32, isOutput=False)
    y_d = nc.declare_dram_parameter("y", [T_C, O_C], F32, isOutput=True)
    scr_d = nc.dram_tensor("wsa_scratch", [2, O_C], F32)

    with tile.TileContext(nc) as tc, contextlib.ExitStack() as ctx:
        xw = ctx.enter_context(tc.tile_pool(name="xw", bufs=4))
        aqp = ctx.enter_context(tc.tile_pool(name="aqp", bufs=2))
        tqp = ctx.enter_context(tc.tile_pool(name="tqp", bufs=3))
        wqp = ctx.enter_context(tc.tile_pool(name="wqp", bufs=2))
        wqt_p = ctx.enter_context(tc.tile_pool(name="wqt", bufs=1))
        bc_p = ctx.enter_context(tc.tile_pool(name="bc", bufs=1))
        sc = ctx.enter_context(tc.tile_pool(name="sc", bufs=8))
        sb_p = ctx.enter_context(tc.tile_pool(name="sb", bufs=3))
        ps = ctx.enter_context(tc.tile_pool(name="ps", bufs=4, space="PSUM"))

        posmagic = bc_p.tile([128, 1], F32, tag="posmagic")
        nc.vector.memset(posmagic, MAGIC)
        negmagic = bc_p.tile([128, 1], F32, tag="negmagic")
        nc.vector.memset(negmagic, -MAGIC)
        hoff = bc_p.tile([128, 1], F32, tag="hoff")
        nc.vector.memset(hoff, HOFF)

        wqt = [
            wqt_p.tile([128, KL, OCW], F16, tag=f"wqt{oc}", name=f"wqt{oc}")
            for oc in range(NOC)
        ]
        bc_wsa = [None] * NOC
        bc_cor = [None] * NOC

        def emit_w(j):
            oc, jj = divmod(j, NJ // NOC)
            wt = xw.tile([128, DI], F32, tag="xw")
            nc.gpsimd.dma_start(out=wt, in_=w_d[j * 128:(j + 1) * 128, :])
            asum = sc.tile([128, 1], F32, tag="asum")
            nc.vector.tensor_reduce(
                out=asum, in_=wt, axis=Ax.X, op=Alu.add,
                apply_absolute_value=True,
            )
            ws = sc.tile([128, 1], F32, tag="ws")
            nc.vector.tensor_scalar(
                out=ws, in0=asum, scalar1=1.0 / DI, scalar2=EPS,
                op0=Alu.mult, op1=Alu.add,
            )
            r = sc.tile([128, 1], F32, tag="wr")
            nc.vector.reciprocal(out=r, in_=ws)
            al_col = sc.tile([128, 1], F32, tag="al")
            nc.gpsimd.dma_start(
                out=al_col,
                in_=al_d[j * 128:(j + 1) * 128].rearrange("(o u) -> o u", u=1),
            )
            wsa = sc.tile([128, 1], F32, tag="wsa")
            nc.vector.tensor_tensor(out=wsa, in0=ws, in1=al_col, op=Alu.mult)
            # magic-round w*r in place, then Sign(x - M) -> exact ternary fp16
            nc.scalar.activation(
                out=wt, in_=wt, func=Act.Identity, bias=posmagic, scale=r
            )
            wq = wqp.tile([128, DI], F16, tag="wq")
            nc.scalar.activation(
                out=wq, in_=wt, func=Act.Sign, bias=negmagic, scale=1.0
            )
            cs = sc.tile([128, 1], F32, tag="cs")
            nc.vector.tensor_reduce(out=cs, in_=wq, axis=Ax.X, op=Alu.add)
            cor = sc.tile([128, 1], F32, tag="cor")
            nc.vector.tensor_scalar_mul(out=cor, in0=cs, scalar1=HOFF)
            nc.gpsimd.dma_start(
                out=bass.AP(tensor=scr_d, offset=j * 128, ap=[[1, 128]]),
                in_=wsa,
            )
            nc.gpsimd.dma_start(
                out=bass.AP(tensor=scr_d, offset=O_C + j * 128, ap=[[1, 128]]),
                in_=cor,
            )
            nc.sync.dma_start_transpose(
                out=wqt[oc][:, :, jj * 128:(jj + 1) * 128], in_=wq
            )
            if jj == NJ // NOC - 1:
                bw = bc_p.tile([128, OCW], F32, tag=f"bcw{oc}")
                nc.gpsimd.dma_start(
                    out=bw,
                    in_=bass.AP(
                        tensor=scr_d, offset=oc * OCW, ap=[[0, 128], [1, OCW]]
                    ),
                )
                bc_wsa[oc] = bw
                bk = bc_p.tile([128, OCW], F32, tag=f"bck{oc}")
                nc.gpsimd.dma_start(
                    out=bk,
                    in_=bass.AP(
                        tensor=scr_d, offset=O_C + oc * OCW,
                        ap=[[0, 128], [1, OCW]],
                    ),
                )
                bc_cor[oc] = bk

        def emit_a(t):
            xt = xw.tile([128, DI], F32, tag="xw")
            nc.gpsimd.dma_start(out=xt, in_=x_d[t * 128:(t + 1) * 128, :])
            amax = sc.tile([128, 1], F32, tag="amax")
            nc.vector.tensor_reduce(
                out=amax, in_=xt, axis=Ax.X, op=Alu.max,
                apply_absolute_value=True,
            )
            s = sc.tile([128, 1], F32, tag="s")
            nc.vector.tensor_scalar_add(out=s, in0=amax, scalar1=EPS)
            ra = sc.tile([128, 1], F32, tag="ra")
            nc.vector.reciprocal(out=ra, in_=s)
            i127 = sc.tile([128, 1], F32, tag="i127")
            nc.vector.tensor_scalar_mul(out=i127, in0=ra, scalar1=QMAX)
            stok = sc.tile([128, 1], F32, tag="stok")
            nc.vector.tensor_scalar_mul(out=stok, in0=s, scalar1=1.0 / QMAX)
            # single-pass quantize: a_scaled + 1536 in [1409,1663] where the
            # fp16 output cast (ulp 1) rounds to the nearest integer -- giving
            # exactly a_quant + 1536; the offset is removed via the colsum
            # correction
            aqh = aqp.tile([128, DI], F16, tag="aq")
            nc.scalar.activation(
                out=aqh, in_=xt, func=Act.Identity, bias=hoff, scale=i127
            )
            aqT = tqp.tile([128, KL, 128], F16, tag="aqT")
            nc.sync.dma_start_transpose(out=aqT, in_=aqh)
            return aqT, stok

        def emit_mm(t, aqT, oc):
            psum = ps.tile([128, OCW], F32, tag=f"psum{oc}", name=f"psum{oc}")
            for kk in range(KL):
                nc.tensor.matmul(
                    psum,
                    lhsT=aqT[:, kk, :],
                    rhs=wqt[oc][:, kk, :],
                    start=(kk == 0),
                    stop=(kk == KL - 1),
                )
            return psum

        def emit_epi(t, psums, stok):
            sbt = sb_p.tile([128, O_C], F32, tag="sb")
            for oc in range(NOC):
                d = sbt[:, oc * OCW:(oc + 1) * OCW]
                nc.vector.tensor_tensor(
                    out=d, in0=psums[oc], in1=bc_cor[oc], op=Alu.subtract
                )
                nc.scalar.activation(
                    out=d, in_=d, func=Act.Copy, bias=0.0, scale=stok
                )
                nc.vector.tensor_tensor(
                    out=d, in0=d, in1=bc_wsa[oc], op=Alu.mult
                )
            nc.gpsimd.dma_start(
                out=y_d[t * 128:(t + 1) * 128, :], in_=sbt
            )

        # Emission order doubles as scheduling priority AND correctness:
        # Tile's dependency tracking is history-based, so an instruction may
        # only read a tile slice whose writer was emitted earlier. W row
        # tiles j=0..3 fill wqt[0], j=4..7 fill wqt[1]; matmuls against
        # wqt[1] and all epilogues (which read the bcast rows written at
        # j=7) are deferred until W(7) has been emitted.
        NW0 = NJ // NOC  # 4: W tiles per output chunk
        for j in range(NW0):
            emit_w(j)
        pend = {}
        for t in range(NW0):
            emit_w(t + NW0)
            aqT, stok = emit_a(t)
            pend[t] = (aqT, stok, emit_mm(t, aqT, oc=0))
        for t in sorted(pend):
            aqT, stok, ps0 = pend[t]
            emit_epi(t, [ps0, emit_mm(t, aqT, oc=1)], stok)
        for t in range(NW0, NT):
            aqT, stok = emit_a(t)
            ps0 = emit_mm(t, aqT, oc=0)
            ps1 = emit_mm(t, aqT, oc=1)
            emit_epi(t, [ps0, ps1], stok)

    _split_sync_waits(nc, mybir, max_waits=1)
    return nc


def _get_nc():
    if "nc" not in _cached:
        _cached["nc"] = _build()
    return _cached["nc"]


def _run(x, weight, alpha, trace=False):
    from concourse.bass_utils import run_bass_kernel_spmd

    nc = _get_nc()
    x_flat = np.ascontiguousarray(np.asarray(x).reshape(B * S, DI))
    weight = np.asarray(weight)
    alpha = np.asarray(alpha)
    in_maps = []
    for c in range(8):
        dp, tp = divmod(c, TP)
        in_maps.append(
            {
                "x": np.ascontiguousarray(x_flat[dp * T_C:(dp + 1) * T_C]),
                "w": np.ascontiguousarray(weight[tp * O_C:(tp + 1) * O_C]),
                "alpha": np.ascontiguousarray(alpha[tp * O_C:(tp + 1) * O_C]),
            }
        )
    res = run_bass_kernel_spmd(nc, in_maps, list(range(8)), trace=trace)
    y = np.empty((B * S, DOUT), np.float32)
    for c in range(8):
        dp, tp = divmod(c, TP)
        y[dp * T_C:(dp + 1) * T_C, tp * O_C:(tp + 1) * O_C] = res.results[c]["y"]
    return y.reshape(B, S, DOUT), res


def kernel(x, weight, alpha):
    y, _ = _run(x, weight, alpha, trace=False)
    return y


# revision 9
# speedup vs baseline: 1.0609x; 1.0609x over previous
"""BitLinear forward on 8 Trainium2 NeuronCores.

Sharding: 2-way data parallel over tokens x 4-way tensor parallel over
output features. Each core computes y[token_half, out_quarter] =
[4096, 1024] of the full [8192, 4096] output.

Per-core pipeline (all on-device), engine-balanced so the TensorE is
the bottleneck:
  W phase (per 128-row tile, 8 tiles): stream W [128, 4096] f32 in one
    DMA; DVE abs-sum reduce -> w_scale; ACT magic-round (scale=1/ws,
    bias=+M) then ACT Sign(x - M) -> exact ternary fp16 (clip(round(v))
    == sign(round(v)) for ternary). DVE reduce -> colsum (for the fp16
    magic-offset correction). w_scale*alpha and 1536*colsum rows bounce
    through DRAM and broadcast-load as [128, 512] tiles. DMA-transpose
    ternary weights into wqt (contraction dim on partitions).
  A phase (per 128-token tile, 32 tiles): one 2 MiB DMA, one DVE
    abs-max reduce -> a_scale, ACT magic-round to a+M (f32), ACT rebias
    by (1536 - M) -> fp16 holding exactly a_quant + 1536 (integers
    <= 1663, exact in fp16 -- the +1536 offset folds the de-magic step
    into the per-output-channel correction). DMA-transpose, then 32
    accumulating fp16 matmuls per 512-wide PSUM chunk.
  Epilogue: DVE subtracts the 1536*colsum broadcast row (exact integer
    y), ACT scales by a_scale/127 per token, DVE multiplies by the
    w_scale*alpha broadcast row; one 0.5 MiB DMA out per token tile.

All wide elementwise ops run on ACT (activation) or DVE tensor_tensor /
tensor_reduce at ~1 elem/cycle; wide tensor_scalar on DVE/GpSimd (~14
cycles/elem on this hw) is avoided entirely. DMA-transpose descriptor
generation (~5us per issue) runs on GpSimd (aq) and Sync (wq), keeping
ACT free.

The quantized operands are exact in fp16 (integers <= 1663, ternary
weights), so the fp16 matmul is bit-exact integer arithmetic.
"""
import sys

sys.path.insert(0, "/opt/trn_rl_repo")

import numpy as np

B, S, DI, DOUT = 4, 2048, 4096, 4096
DP, TP = 2, 4
T_C = B * S // DP      # 4096 tokens per core
O_C = DOUT // TP       # 1024 out features per core
NT = T_C // 128        # 32 token tiles
NJ = O_C // 128        # 8 weight row tiles
KL = DI // 128         # 32 k-slices
OCW = 512              # output chunk width (one PSUM bank)
NOC = O_C // OCW       # 2 output chunks

EPS = 1e-8
QMAX = 127.0
MAGIC = 12582912.0     # 1.5 * 2**23; f32 add rounds to nearest-even int
HOFF = 1536.0          # fp16 offset: a+1536 in [1409,1663], exact in fp16

_cached = {}


def _install_walrus_workarounds(tile_mod, mybir):
    """This walrus build rejects instructions with more than one sem wait
    ('Too many sync wait commands'). Split the Tile tail-drain waits over
    several sequencer drains; regular instructions are handled by
    _split_sync_waits after scheduling."""
    from concourse.vector_clock import ScopedClock

    def _drain_and_barrier_split(self, tick_clock, wait_clock):
        drain_inst = self.nc.sync.drain()
        wait_clock.add_sem_waits(
            drain_inst.ins, ScopedClock({None: tick_clock.global_clock})
        )
        waits = list(drain_inst.ins.sync_info.on_wait)
        if len(waits) > 1:
            del drain_inst.ins.sync_info.on_wait[1:]
            for w in waits[1:]:
                extra = self.nc.sync.drain()
                extra.ins.sync_info = mybir.SyncInfo(on_wait=[w], on_update=[])

        self.nc.all_engine_barrier()
        assert self.sems is not None
        popped = self.nc._tile_sem_poison_stack.pop()
        assert popped is self._sem_poison
        self.nc.clear_and_free_semaphores(list(self.sems.allocated().values()))
        self.nc.all_engine_barrier()

    tile_mod.TileContext._drain_and_barrier = _drain_and_barrier_split


def _split_sync_waits(nc, mybir, max_waits=1):
    """Move excess sem waits onto same-engine NoOps inserted before the
    offending instruction (engines run their stream in order, so the wait
    conjunction is preserved)."""
    n = 0
    for fn in nc.m.functions:
        for bb in fn.blocks:
            insts = bb.instructions
            i = 0
            while i < len(insts):
                inst = insts[i]
                si = getattr(inst, "sync_info", None)
                if si is not None and si.on_wait and len(si.on_wait) > max_waits:
                    waits = list(si.on_wait)
                    extra = waits[: len(waits) - max_waits]
                    del si.on_wait[: len(waits) - max_waits]
                    nops = []
                    for j in range(0, len(extra), max_waits):
                        nop = mybir.InstNoOp(name=f"WSPLIT-{n}", ins=[], outs=[])
                        n += 1
                        nop.engine = inst.engine
                        nop.sync_info = mybir.SyncInfo(
                            on_wait=list(extra[j : j + max_waits]), on_update=[]
                        )
                        nops.append(nop)
                    insts[i:i] = nops
                    i += len(nops)
                i += 1
    return n


def _build():
    import contextlib

    import concourse.bass as bass
    import concourse.tile as tile
    from concourse import mybir

    _install_walrus_workarounds(tile, mybir)

    F32 = mybir.dt.float32
    F16 = mybir.dt.float16
    Alu = mybir.AluOpType
    Act = mybir.ActivationFunctionType
    Ax = mybir.AxisListType

    nc = bass.Bass("TRN2", target_bir_lowering=False, debug=False, num_devices=8)
    x_d = nc.declare_dram_parameter("x", [T_C, DI], F32, isOutput=False)
    w_d = nc.declare_dram_parameter("w", [O_C, DI], F32, isOutput=False)
    al_d = nc.declare_dram_parameter("alpha", [O_C], F32, isOutput=False)
    y_d = nc.declare_dram_parameter("y", [T_C, O_C], F32, isOutput=True)
    scr_d = nc.dram_tensor("wsa_scratch", [O_C], F32)

    with tile.TileContext(nc) as tc, contextlib.ExitStack() as ctx:
        xw = ctx.enter_context(tc.tile_pool(name="xw", bufs=4))
        aqp = ctx.enter_context(tc.tile_pool(name="aqp", bufs=2))
        tqp = ctx.enter_context(tc.tile_pool(name="tqp", bufs=4))
        wqp = ctx.enter_context(tc.tile_pool(name="wqp", bufs=1))
        wqt_p = ctx.enter_context(tc.tile_pool(name="wqt", bufs=1))
        bc_p = ctx.enter_context(tc.tile_pool(name="bc", bufs=1))
        sc = ctx.enter_context(tc.tile_pool(name="sc", bufs=8))
        sb_p = ctx.enter_context(tc.tile_pool(name="sb", bufs=2))
        ps = ctx.enter_context(tc.tile_pool(name="ps", bufs=4, space="PSUM"))

        posmagic = bc_p.tile([128, 1], F32, tag="posmagic")
        nc.vector.memset(posmagic, MAGIC)
        negmagic = bc_p.tile([128, 1], F32, tag="negmagic")
        nc.vector.memset(negmagic, -MAGIC)
        hoff = bc_p.tile([128, 1], F32, tag="hoff")
        nc.vector.memset(hoff, HOFF)
        ones = bc_p.tile([128, 128], F16, tag="ones")
        nc.vector.memset(ones, 1.0)

        wqt = [
            wqt_p.tile([128, KL, OCW], F16, tag=f"wqt{oc}", name=f"wqt{oc}")
            for oc in range(NOC)
        ]
        bc_wsa = [None] * NOC
        bc_cor = [None] * NOC

        def emit_w(j):
            oc, jj = divmod(j, NJ // NOC)
            wt = xw.tile([128, DI], F32, tag="xw")
            nc.gpsimd.dma_start(out=wt, in_=w_d[j * 128:(j + 1) * 128, :])
            asum = sc.tile([128, 1], F32, tag="asum")
            nc.vector.tensor_reduce(
                out=asum, in_=wt, axis=Ax.X, op=Alu.add,
                apply_absolute_value=True,
            )
            ws = sc.tile([128, 1], F32, tag="ws")
            nc.vector.tensor_scalar(
                out=ws, in0=asum, scalar1=1.0 / DI, scalar2=EPS,
                op0=Alu.mult, op1=Alu.add,
            )
            r = sc.tile([128, 1], F32, tag="wr")
            nc.vector.reciprocal(out=r, in_=ws)
            al_col = sc.tile([128, 1], F32, tag="al")
            nc.gpsimd.dma_start(
                out=al_col,
                in_=al_d[j * 128:(j + 1) * 128].rearrange("(o u) -> o u", u=1),
            )
            wsa = sc.tile([128, 1], F32, tag="wsa")
            nc.vector.tensor_tensor(out=wsa, in0=ws, in1=al_col, op=Alu.mult)
            # magic-round w*r in place, then Sign(x - M) -> exact ternary fp16
            nc.scalar.activation(
                out=wt, in_=wt, func=Act.Identity, bias=posmagic, scale=r
            )
            wq = wqp.tile([128, DI], F16, tag="wq")
            nc.scalar.activation(
                out=wq, in_=wt, func=Act.Sign, bias=negmagic, scale=1.0
            )
            nc.gpsimd.dma_start(
                out=bass.AP(tensor=scr_d, offset=j * 128, ap=[[1, 128]]),
                in_=wsa,
            )
            nc.sync.dma_start_transpose(
                out=wqt[oc][:, :, jj * 128:(jj + 1) * 128], in_=wq
            )
            if jj == NJ // NOC - 1:
                bw = bc_p.tile([128, OCW], F32, tag=f"bcw{oc}")
                nc.gpsimd.dma_start(
                    out=bw,
                    in_=bass.AP(
                        tensor=scr_d, offset=oc * OCW, ap=[[0, 128], [1, OCW]]
                    ),
                )
                bc_wsa[oc] = bw
                # colsum on TensorE: ones.T @ wqt accumulates sum_k w[k, o]
                # into every psum partition -- the broadcast tile directly
                pcs = ps.tile([128, OCW], F32, tag="psum1", name="pscs")
                for kk in range(KL):
                    nc.tensor.matmul(
                        pcs,
                        lhsT=ones,
                        rhs=wqt[oc][:, kk, :],
                        start=(kk == 0),
                        stop=(kk == KL - 1),
                    )
                bk = bc_p.tile([128, OCW], F32, tag=f"bck{oc}")
                nc.scalar.activation(
                    out=bk, in_=pcs, func=Act.Copy, bias=0.0, scale=HOFF
                )
                bc_cor[oc] = bk

        def emit_a(t):
            xt = xw.tile([128, DI], F32, tag="xw")
            nc.gpsimd.dma_start(out=xt, in_=x_d[t * 128:(t + 1) * 128, :])
            amax = sc.tile([128, 1], F32, tag="amax")
            nc.vector.tensor_reduce(
                out=amax, in_=xt, axis=Ax.X, op=Alu.max,
                apply_absolute_value=True,
            )
            s = sc.tile([128, 1], F32, tag="s")
            nc.vector.tensor_scalar_add(out=s, in0=amax, scalar1=EPS)
            ra = sc.tile([128, 1], F32, tag="ra")
            nc.vector.reciprocal(out=ra, in_=s)
            i127 = sc.tile([128, 1], F32, tag="i127")
            nc.vector.tensor_scalar_mul(out=i127, in0=ra, scalar1=QMAX)
            stok = sc.tile([128, 1], F32, tag="stok")
            nc.vector.tensor_scalar_mul(out=stok, in0=s, scalar1=1.0 / QMAX)
            # single-pass quantize: a_scaled + 1536 in [1409,1663] where the
            # fp16 output cast (ulp 1) rounds to the nearest integer -- giving
            # exactly a_quant + 1536; the offset is removed via the colsum
            # correction
            aqh = aqp.tile([128, DI], F16, tag="aq")
            nc.scalar.activation(
                out=aqh, in_=xt, func=Act.Identity, bias=hoff, scale=i127
            )
            aqT = tqp.tile([128, KL, 128], F16, tag="aqT")
            nc.sync.dma_start_transpose(out=aqT, in_=aqh)
            return aqT, stok

        def emit_mm(t, aqT, oc):
            psum = ps.tile([128, OCW], F32, tag=f"psum{oc}", name=f"psum{oc}")
            for kk in range(KL):
                nc.tensor.matmul(
                    psum,
                    lhsT=aqT[:, kk, :],
                    rhs=wqt[oc][:, kk, :],
                    start=(kk == 0),
                    stop=(kk == KL - 1),
                )
            return psum

        def emit_epi(t, psums, stok):
            sbt = sb_p.tile([128, O_C], F32, tag="sb")
            for oc in range(NOC):
                d = sbt[:, oc * OCW:(oc + 1) * OCW]
                nc.vector.tensor_tensor(
                    out=d, in0=psums[oc], in1=bc_cor[oc], op=Alu.subtract
                )
                nc.scalar.activation(
                    out=d, in_=d, func=Act.Copy, bias=0.0, scale=stok
                )
                nc.vector.tensor_tensor(
                    out=d, in0=d, in1=bc_wsa[oc], op=Alu.mult
                )
            nc.gpsimd.dma_start(
                out=y_d[t * 128:(t + 1) * 128, :], in_=sbt
            )

        # Emission order doubles as scheduling priority AND correctness:
        # Tile's dependency tracking is history-based, so an instruction may
        # only read a tile slice whose writer was emitted earlier. W row
        # tiles j=0..3 fill wqt[0], j=4..7 fill wqt[1]; matmuls against
        # wqt[1] and all epilogues (which read the bcast rows written at
        # j=7) are deferred until W(7) has been emitted.
        NW0 = NJ // NOC  # 4: W tiles per output chunk
        for j in range(NW0):
            emit_w(j)
        pend = {}
        for t in range(NW0):
            aqT, stok = emit_a(t)
            emit_w(t + NW0)
            pend[t] = (aqT, stok, emit_mm(t, aqT, oc=0))
        for t in sorted(pend):
            aqT, stok, ps0 = pend[t]
            emit_epi(t, [ps0, emit_mm(t, aqT, oc=1)], stok)
        for t in range(NW0, NT):
            aqT, stok = emit_a(t)
            ps0 = emit_mm(t, aqT, oc=0)
            ps1 = emit_mm(t, aqT, oc=1)
            emit_epi(t, [ps0, ps1], stok)

    _split_sync_waits(nc, mybir, max_waits=1)
    return nc


def _get_nc():
    if "nc" not in _cached:
        _cached["nc"] = _build()
    return _cached["nc"]


def _run(x, weight, alpha, trace=False):
    from concourse.bass_utils import run_bass_kernel_spmd

    nc = _get_nc()
    x_flat = np.ascontiguousarray(np.asarray(x).reshape(B * S, DI))
    weight = np.asarray(weight)
    alpha = np.asarray(alpha)
    in_maps = []
    for c in range(8):
        dp, tp = divmod(c, TP)
        in_maps.append(
            {
                "x": np.ascontiguousarray(x_flat[dp * T_C:(dp + 1) * T_C]),
                "w": np.ascontiguousarray(weight[tp * O_C:(tp + 1) * O_C]),
                "alpha": np.ascontiguousarray(alpha[tp * O_C:(tp + 1) * O_C]),
            }
        )
    res = run_bass_kernel_spmd(nc, in_maps, list(range(8)), trace=trace)
    y = np.empty((B * S, DOUT), np.float32)
    for c in range(8):
        dp, tp = divmod(c, TP)
        y[dp * T_C:(dp + 1) * T_C, tp * O_C:(tp + 1) * O_C] = res.results[c]["y"]
    return y.reshape(B, S, DOUT), res


def kernel(x, weight, alpha):
    y, _ = _run(x, weight, alpha, trace=False)
    return y
